# revision 2
# baseline (speedup 1.0000x reference)
"""Trainium2 Bass kernel for BatchChannelDecorrelationLoss.

Contract: kernel(**inputs) takes FULL unsharded inputs
  y:             (16, 192, 32, 32) f32
  x_hat:         (16, 3, 512, 512) f32
  target:        (16, 3, 512, 512) f32
  likelihoods_y: (16, 192, 32, 32) f32
and returns the FULL output: scalar f32 loss.

Strategy (data-parallel over batch N across 8 cores, 2 samples/core):
  device, per core:
    - per-(n,c) max / min / sum of y over H*W          -> stats (384, 3)
    - full-channel Gram  G = Y^T Y  over its 2048 rows -> gram  (192, 192)
    - sum((x_hat-target)^2) partial per partition      -> red[:, 0]
    - sum(log(lik)) partial per partition              -> red[:, 1]
  host:
    - rates = sum_n (round(max) - round(min))  [round commutes with max/min]
    - stable argsort -> top-64 channel idx  (matches jnp.argsort tie-break)
    - cov = (G_k - S_k S_k^T / M) / (M-1) on the selected 64x64 block
    - loss = lmbda*255^2*mse + bpp + lmbda_corr*sum(offdiag(cov)^2)
"""

import math
import sys
import types

if "/opt/trn_rl_repo" not in sys.path:
    sys.path.insert(0, "/opt/trn_rl_repo")

import numpy as np

import concourse.bass as bass
import concourse.bacc as bacc
import concourse.masks as masks
import concourse.mybir as mybir
import concourse.tile as tile
from concourse.bass_utils import run_bass_kernel_spmd

# ---- problem constants (hardcoded per spec) ----
N, C, HY, WY = 16, 192, 32, 32
NI, CI, HI, WI = 16, 3, 512, 512
TOP_K = 64
LMBDA = 0.01
LMBDA_CORR = 1e-4
N_CORES = 8
NS = N // N_CORES          # samples per core = 2
YROWS = NS * C             # 384
YCOLS = HY * WY            # 1024
MSE_COLS = NS * CI * HI * WI // 128   # 12288
LIK_COLS = NS * C * HY * WY // 128    # 3072
MSE_CHUNK = 2048
N_MSE = MSE_COLS // MSE_CHUNK         # 6

FP32 = mybir.dt.float32
AX = mybir.AxisListType
OP = mybir.AluOpType
AF = mybir.ActivationFunctionType

_prog_cache = {}


def _build_program():
    nc = bacc.Bacc("TRN2", target_bir_lowering=False, debug=False,
                   num_devices=N_CORES)

    ys = nc.dram_tensor("ys", [YROWS, YCOLS], FP32, kind="ExternalInput")
    xh = nc.dram_tensor("xh", [128, MSE_COLS], FP32, kind="ExternalInput")
    tg = nc.dram_tensor("tg", [128, MSE_COLS], FP32, kind="ExternalInput")
    lk = nc.dram_tensor("lk", [128, LIK_COLS], FP32, kind="ExternalInput")

    stats = nc.dram_tensor("stats", [YROWS, 3], FP32, kind="ExternalOutput")
    gram = nc.dram_tensor("gram", [C, C], FP32, kind="ExternalOutput")
    red = nc.dram_tensor("red", [128, 2], FP32, kind="ExternalOutput")

    with tile.TileContext(nc) as tc:
        with (
            tc.tile_pool(name="singles", bufs=1) as singles,
            tc.tile_pool(name="ypool", bufs=3) as ypool,
            tc.tile_pool(name="ytp", bufs=16) as ytp,
            tc.tile_pool(name="stp", bufs=3) as stp,
            tc.tile_pool(name="mx", bufs=3) as mxp,
            tc.tile_pool(name="mt", bufs=3) as mtp,
            tc.tile_pool(name="msc", bufs=2) as mscp,
            tc.tile_pool(name="lkp", bufs=1) as lkp,
            tc.tile_pool(name="tpsum", bufs=4, space="PSUM") as tpsum,
            tc.tile_pool(name="gpsum", bufs=1, space="PSUM") as gpsum,
        ):
            ident = singles.tile([128, 128], FP32)
            masks.make_identity(nc, ident[:])

            # ---- y: load 3 row-tiles, per-row max/min/sum ----
            ytiles = []
            for t in range(3):
                yt = ypool.tile([128, YCOLS], FP32, tag="yt")
                nc.sync.dma_start(yt[:], ys[t * 128:(t + 1) * 128, :])
                ytiles.append(yt)
                st = stp.tile([128, 3], FP32, tag="st")
                nc.vector.tensor_reduce(st[:, 0:1], yt[:], axis=AX.X, op=OP.max)
                nc.vector.tensor_reduce(st[:, 1:2], yt[:], axis=AX.X, op=OP.min)
                nc.vector.tensor_reduce(st[:, 2:3], yt[:], axis=AX.X, op=OP.add)
                nc.sync.dma_start(stats[t * 128:(t + 1) * 128, :], st[:])

            # ---- transpose y to (hw, c) tiles: 16 tiles of (128, 192) ----
            # sample 0: c0..127 = ytiles[0][:],   c128..191 = ytiles[1][0:64]
            # sample 1: c0..63  = ytiles[1][64:], c64..191  = ytiles[2][:]
            yts = []
            for s in range(NS):
                for j in range(YCOLS // 128):
                    sl = slice(j * 128, (j + 1) * 128)
                    yt = ytp.tile([128, C], FP32, tag="ytt")
                    if s == 0:
                        pa = tpsum.tile([128, 128], FP32, tag="tp")
                        nc.tensor.transpose(pa[:], ytiles[0][:, sl], ident[:])
                        nc.scalar.copy(yt[:, 0:128], pa[:])
                        pb = tpsum.tile([128, 64], FP32, tag="tp")
                        nc.tensor.transpose(pb[:], ytiles[1][0:64, sl],
                                            ident[0:64, 0:64])
                        nc.scalar.copy(yt[:, 128:192], pb[:])
                    else:
                        pa = tpsum.tile([128, 64], FP32, tag="tp")
                        nc.tensor.transpose(pa[:], ytiles[1][64:128, sl],
                                            ident[64:128, 64:128])
                        nc.scalar.copy(yt[:, 0:64], pa[:])
                        pb = tpsum.tile([128, 128], FP32, tag="tp")
                        nc.tensor.transpose(pb[:], ytiles[2][:, sl], ident[:])
                        nc.scalar.copy(yt[:, 64:192], pb[:])
                    yts.append(yt)

            # ---- Gram: G = sum_k Yt_k^T-contracted products ----
            ga = gpsum.tile([128, C], FP32, tag="ga")
            for k, yt in enumerate(yts):
                nc.tensor.matmul(ga[:], lhsT=yt[:, 0:128], rhs=yt[:],
                                 start=(k == 0), stop=(k == len(yts) - 1))
            gb = gpsum.tile([64, C], FP32, tag="gb")
            for k, yt in enumerate(yts):
                nc.tensor.matmul(gb[:], lhsT=yt[:, 128:192], rhs=yt[:],
                                 start=(k == 0), stop=(k == len(yts) - 1))
            gasb = singles.tile([128, C], FP32)
            nc.vector.tensor_copy(gasb[:], ga[:])
            nc.sync.dma_start(gram[0:128, :], gasb[:])
            gbsb = singles.tile([64, C], FP32)
            nc.vector.tensor_copy(gbsb[:], gb[:])
            nc.sync.dma_start(gram[128:192, :], gbsb[:])

            # ---- MSE: sum((xh - tg)^2), partials per partition ----
            macc = singles.tile([128, N_MSE], FP32)
            for i in range(N_MSE):
                sl = slice(i * MSE_CHUNK, (i + 1) * MSE_CHUNK)
                xt = mxp.tile([128, MSE_CHUNK], FP32, tag="xt")
                nc.sync.dma_start(xt[:], xh[:, sl])
                tt = mtp.tile([128, MSE_CHUNK], FP32, tag="tt")
                nc.sync.dma_start(tt[:], tg[:, sl])
                nc.vector.tensor_tensor(xt[:], xt[:], tt[:], op=OP.subtract)
                sq = mscp.tile([128, MSE_CHUNK], FP32, tag="sq")
                nc.scalar.activation(sq[:], xt[:], AF.Square,
                                     accum_out=macc[:, i:i + 1])

            redsb = singles.tile([128, 2], FP32)
            nc.vector.tensor_reduce(redsb[:, 0:1], macc[:], axis=AX.X, op=OP.add)

            # ---- likelihoods: sum(log(lk)) partials per partition ----
            lt = lkp.tile([128, LIK_COLS], FP32)
            nc.sync.dma_start(lt[:], lk[:])
            lnout = lkp.tile([128, LIK_COLS], FP32)
            nc.scalar.activation(lnout[:], lt[:], AF.Ln,
                                 accum_out=redsb[:, 1:2])
            nc.sync.dma_start(red[:], redsb[:])

    nc.compile()
    return nc


def _get_program():
    if "nc" not in _prog_cache:
        _prog_cache["nc"] = _build_program()
    return _prog_cache["nc"]


def kernel(y, x_hat, target, likelihoods_y):
    y = np.ascontiguousarray(y, dtype=np.float32)
    x_hat = np.ascontiguousarray(x_hat, dtype=np.float32)
    target = np.ascontiguousarray(target, dtype=np.float32)
    lik = np.ascontiguousarray(likelihoods_y, dtype=np.float32)

    nc = _get_program()

    in_maps = []
    for c in range(N_CORES):
        s = slice(c * NS, (c + 1) * NS)
        in_maps.append({
            "ys": y[s].reshape(YROWS, YCOLS),
            "xh": x_hat[s].reshape(128, MSE_COLS),
            "tg": target[s].reshape(128, MSE_COLS),
            "lk": lik[s].reshape(128, LIK_COLS),
        })

    res = run_bass_kernel_spmd(nc, in_maps, list(range(N_CORES)))
    results = res.results

    # ---- host-side combine (all O(C^2) and smaller) ----
    stats = np.stack([r["stats"] for r in results])       # (8, 384, 3)
    stats = stats.reshape(N_CORES, NS, C, 3).reshape(N, C, 3)
    fmax, fmin, fsum = stats[..., 0], stats[..., 1], stats[..., 2]

    # rates: round commutes with max/min; np.round == jnp.round (half-to-even)
    per_sample = np.round(fmax).astype(np.int64) - np.round(fmin).astype(np.int64)
    rates = per_sample.sum(axis=0)                        # (192,)
    idx = np.argsort(rates, kind="stable")[::-1][:TOP_K]

    G = np.sum([r["gram"] for r in results], axis=0, dtype=np.float64)
    S = fsum.astype(np.float64).sum(axis=0)               # (192,)
    M = N * HY * WY                                       # 16384
    Gk = G[np.ix_(idx, idx)]
    Sk = S[idx]
    cov = (Gk - np.outer(Sk, Sk) / M) / (M - 1)
    off = cov - np.diag(np.diag(cov))
    corr_loss = float(np.sum(off ** 2))

    red = np.sum([r["red"] for r in results], axis=0, dtype=np.float64)  # (128,2)
    mse_sum = float(red[:, 0].sum())
    ln_sum = float(red[:, 1].sum())

    num_pixels = N * HI * WI
    mse_loss = mse_sum / (NI * CI * HI * WI)
    bpp_loss = ln_sum / (-math.log(2) * num_pixels)
    loss = LMBDA * 255.0 ** 2 * mse_loss + bpp_loss + LMBDA_CORR * corr_loss
    return np.float32(loss)


# revision 3
# speedup vs baseline: 1.1320x; 1.1320x over previous
"""Trainium2 Bass kernel for BatchChannelDecorrelationLoss.

Contract: kernel(**inputs) takes FULL unsharded inputs
  y:             (16, 192, 32, 32) f32
  x_hat:         (16, 3, 512, 512) f32
  target:        (16, 3, 512, 512) f32
  likelihoods_y: (16, 192, 32, 32) f32
and returns the FULL output: scalar f32 loss.

Strategy (data-parallel over batch N across 8 cores, 2 samples/core):
  device, per core:
    - per-(n,c) max / min of y over H*W (f32, exact)   -> stats (384, 2)
    - row-Gram B = Z^T Z over all 384 (n,c) rows, bf16 -> b0/b1/b2 tiles
      (upper block-triangle; host extracts the two per-sample 192x192
       diagonal blocks; bf16 is fine: corr term is ~1e-6 of the loss)
    - row sums via ones-vector matmul                  -> rs (1, 384)
    - sum((x_hat-target)^2) partial per partition      -> red[:, 0]
    - sum(log(lik)) partial per partition              -> red[:, 1]
  host:
    - rates = sum_n (round(max) - round(min))  [round commutes with max/min]
    - stable argsort -> top-64 channel idx  (matches jnp.argsort tie-break)
    - cov = (G_k - S_k S_k^T / M) / (M-1) on the selected 64x64 block
    - loss = lmbda*255^2*mse + bpp + lmbda_corr*sum(offdiag(cov)^2)

DMA layout: all heavy loads issued first (y, lik, then x_hat on the sync
HWDGE queue and target on the scalar HWDGE queue); all stores go through
the gpsimd SWDGE queue so they can never head-of-line-block the loads.
"""

import math
import sys

if "/opt/trn_rl_repo" not in sys.path:
    sys.path.insert(0, "/opt/trn_rl_repo")

import numpy as np

import concourse.bacc as bacc
import concourse.masks as masks
import concourse.mybir as mybir
import concourse.tile as tile
from concourse.bass_utils import run_bass_kernel_spmd

# ---- problem constants (hardcoded per spec) ----
N, C, HY, WY = 16, 192, 32, 32
NI, CI, HI, WI = 16, 3, 512, 512
TOP_K = 64
LMBDA = 0.01
LMBDA_CORR = 1e-4
N_CORES = 8
NS = N // N_CORES          # samples per core = 2
YROWS = NS * C             # 384
YCOLS = HY * WY            # 1024
MSE_COLS = NS * CI * HI * WI // 128   # 12288
LIK_COLS = NS * C * HY * WY // 128    # 3072
MSE_CHUNK = 2048
N_MSE = MSE_COLS // MSE_CHUNK         # 6
NJ = YCOLS // 128                     # 8 hw chunks

FP32 = mybir.dt.float32
BF16 = mybir.dt.bfloat16
AX = mybir.AxisListType
OP = mybir.AluOpType
AF = mybir.ActivationFunctionType

_prog_cache = {}


def _build_program():
    nc = bacc.Bacc("TRN2", target_bir_lowering=False, debug=False,
                   num_devices=N_CORES)

    ys = nc.dram_tensor("ys", [YROWS, YCOLS], FP32, kind="ExternalInput")
    xh = nc.dram_tensor("xh", [128, MSE_COLS], FP32, kind="ExternalInput")
    tg = nc.dram_tensor("tg", [128, MSE_COLS], FP32, kind="ExternalInput")
    lk = nc.dram_tensor("lk", [128, LIK_COLS], FP32, kind="ExternalInput")

    stats = nc.dram_tensor("stats", [YROWS, 2], FP32, kind="ExternalOutput")
    b0 = nc.dram_tensor("b0", [128, 256], FP32, kind="ExternalOutput")
    b1 = nc.dram_tensor("b1", [128, 256], FP32, kind="ExternalOutput")
    b2 = nc.dram_tensor("b2", [128, 128], FP32, kind="ExternalOutput")
    rs = nc.dram_tensor("rs", [1, YROWS], FP32, kind="ExternalOutput")
    red = nc.dram_tensor("red", [128, 2], FP32, kind="ExternalOutput")

    with tile.TileContext(nc) as tc:
        with (
            tc.tile_pool(name="singles", bufs=1) as singles,
            tc.tile_pool(name="ypool", bufs=3) as ypool,
            tc.tile_pool(name="ybf", bufs=3) as ybfp,
            tc.tile_pool(name="ztp", bufs=8) as ztp,
            tc.tile_pool(name="stp", bufs=3) as stp,
            tc.tile_pool(name="mx", bufs=3) as mxp,
            tc.tile_pool(name="mt", bufs=3) as mtp,
            tc.tile_pool(name="msc", bufs=2) as mscp,
            tc.tile_pool(name="lkp", bufs=1) as lkp,
            tc.tile_pool(name="tpsum", bufs=4, space="PSUM") as tpsum,
            tc.tile_pool(name="gpsum", bufs=1, space="PSUM") as gpsum,
        ):
            # ---- phase 0: all heavy loads, in issue order ----
            ytiles = []
            for t in range(3):
                yt = ypool.tile([128, YCOLS], FP32, tag="yt")
                nc.sync.dma_start(yt[:], ys[t * 128:(t + 1) * 128, :])
                ytiles.append(yt)
            lt = lkp.tile([128, LIK_COLS], FP32)
            nc.sync.dma_start(lt[:], lk[:])
            mse_x, mse_t = [], []
            for i in range(N_MSE):
                sl = slice(i * MSE_CHUNK, (i + 1) * MSE_CHUNK)
                xt = mxp.tile([128, MSE_CHUNK], FP32, tag="xt")
                nc.sync.dma_start(xt[:], xh[:, sl])
                tt = mtp.tile([128, MSE_CHUNK], FP32, tag="tt")
                nc.scalar.dma_start(tt[:], tg[:, sl])
                mse_x.append(xt)
                mse_t.append(tt)

            ident = singles.tile([128, 128], BF16)
            masks.make_identity(nc, ident[:])
            ones = singles.tile([128, 1], BF16)
            nc.gpsimd.memset(ones[:], 1.0)

            # ---- likelihoods: sum(log(lk)) partials per partition ----
            redsb = singles.tile([128, 2], FP32)
            lnout = lkp.tile([128, LIK_COLS], FP32)
            nc.scalar.activation(lnout[:], lt[:], AF.Ln,
                                 accum_out=redsb[:, 1:2])

            # ---- y: stats + bf16 cast ----
            ybf = []
            for t in range(3):
                st = stp.tile([128, 2], FP32, tag="st")
                nc.vector.tensor_reduce(st[:, 0:1], ytiles[t][:], axis=AX.X,
                                        op=OP.max)
                nc.vector.tensor_reduce(st[:, 1:2], ytiles[t][:], axis=AX.X,
                                        op=OP.min)
                nc.gpsimd.dma_start(stats[t * 128:(t + 1) * 128, :], st[:])
                yb = ybfp.tile([128, YCOLS], BF16, tag="yb")
                nc.vector.tensor_copy(yb[:], ytiles[t][:])
                ybf.append(yb)

            # ---- transpose to Z tiles: 8 x (128 hw, 384 rows) bf16 ----
            zts = []
            for j in range(NJ):
                sl = slice(j * 128, (j + 1) * 128)
                zt = ztp.tile([128, YROWS], BF16, tag="zt")
                for t in range(3):
                    pt = tpsum.tile([128, 128], BF16, tag="tp")
                    nc.tensor.transpose(pt[:], ybf[t][:, sl], ident[:])
                    if t % 2 == 0:
                        nc.scalar.copy(zt[:, t * 128:(t + 1) * 128], pt[:])
                    else:
                        nc.vector.tensor_copy(zt[:, t * 128:(t + 1) * 128],
                                              pt[:])
                zts.append(zt)

            # ---- row-Gram upper blocks + row sums, PSUM-accumulated ----
            pb0 = gpsum.tile([128, 256], FP32, tag="pb0")
            for j, zt in enumerate(zts):
                nc.tensor.matmul(pb0[:], lhsT=zt[:, 0:128], rhs=zt[:, 0:256],
                                 start=(j == 0), stop=(j == NJ - 1))
            pb1 = gpsum.tile([128, 256], FP32, tag="pb1")
            for j, zt in enumerate(zts):
                nc.tensor.matmul(pb1[:], lhsT=zt[:, 128:256],
                                 rhs=zt[:, 128:384],
                                 start=(j == 0), stop=(j == NJ - 1))
            pb2 = gpsum.tile([128, 128], FP32, tag="pb2")
            for j, zt in enumerate(zts):
                nc.tensor.matmul(pb2[:], lhsT=zt[:, 256:384],
                                 rhs=zt[:, 256:384],
                                 start=(j == 0), stop=(j == NJ - 1))
            prs = gpsum.tile([1, YROWS], FP32, tag="prs")
            for j, zt in enumerate(zts):
                nc.tensor.matmul(prs[:], lhsT=ones[:], rhs=zt[:],
                                 start=(j == 0), stop=(j == NJ - 1))

            for psum_t, dram_t, w in ((pb0, b0, 256), (pb1, b1, 256),
                                      (pb2, b2, 128)):
                sb = singles.tile([128, w], FP32, tag=f"sb{w}",
                                  name=f"gout_{dram_t.name}")
                nc.vector.tensor_copy(sb[:], psum_t[:])
                nc.gpsimd.dma_start(dram_t[:], sb[:])
            rssb = singles.tile([1, YROWS], FP32)
            nc.vector.tensor_copy(rssb[:], prs[:])
            nc.gpsimd.dma_start(rs[:], rssb[:])

            # ---- MSE: sum((xh - tg)^2), partials per partition ----
            macc = singles.tile([128, N_MSE], FP32)
            for i in range(N_MSE):
                xt, tt = mse_x[i], mse_t[i]
                nc.vector.tensor_tensor(xt[:], xt[:], tt[:], op=OP.subtract)
                sq = mscp.tile([128, MSE_CHUNK], FP32, tag="sq")
                nc.scalar.activation(sq[:], xt[:], AF.Square,
                                     accum_out=macc[:, i:i + 1])

            nc.vector.tensor_reduce(redsb[:, 0:1], macc[:], axis=AX.X,
                                    op=OP.add)
            nc.gpsimd.dma_start(red[:], redsb[:])

    nc.compile()
    return nc


def _get_program():
    if "nc" not in _prog_cache:
        _prog_cache["nc"] = _build_program()
    return _prog_cache["nc"]


def kernel(y, x_hat, target, likelihoods_y):
    y = np.ascontiguousarray(y, dtype=np.float32)
    x_hat = np.ascontiguousarray(x_hat, dtype=np.float32)
    target = np.ascontiguousarray(target, dtype=np.float32)
    lik = np.ascontiguousarray(likelihoods_y, dtype=np.float32)

    nc = _get_program()

    in_maps = []
    for c in range(N_CORES):
        s = slice(c * NS, (c + 1) * NS)
        in_maps.append({
            "ys": y[s].reshape(YROWS, YCOLS),
            "xh": x_hat[s].reshape(128, MSE_COLS),
            "tg": target[s].reshape(128, MSE_COLS),
            "lk": lik[s].reshape(128, LIK_COLS),
        })

    res = run_bass_kernel_spmd(nc, in_maps, list(range(N_CORES)))
    results = res.results

    # ---- host-side combine (all O(C^2) and smaller) ----
    stats = np.stack([r["stats"] for r in results])       # (8, 384, 2)
    stats = stats.reshape(N, C, 2)
    fmax, fmin = stats[..., 0], stats[..., 1]

    # rates: round commutes with max/min; np.round == jnp.round (half-to-even)
    per_sample = np.round(fmax).astype(np.int64) - np.round(fmin).astype(np.int64)
    rates = per_sample.sum(axis=0)                        # (192,)
    idx = np.argsort(rates, kind="stable")[::-1][:TOP_K]

    # row-Gram -> per-channel Gram G and sums S
    B = np.zeros((YROWS, YROWS), dtype=np.float64)
    for r in results:
        B[0:128, 0:256] += r["b0"]
        B[128:256, 128:384] += r["b1"]
        B[256:384, 256:384] += r["b2"]
    B = np.triu(B) + np.triu(B, 1).T
    G = B[0:C, 0:C] + B[C:2 * C, C:2 * C]

    rs_all = np.sum([r["rs"] for r in results], axis=0,
                    dtype=np.float64).reshape(YROWS)
    S = rs_all[0:C] + rs_all[C:2 * C]

    M = N * HY * WY                                       # 16384
    Gk = G[np.ix_(idx, idx)]
    Sk = S[idx]
    cov = (Gk - np.outer(Sk, Sk) / M) / (M - 1)
    off = cov - np.diag(np.diag(cov))
    corr_loss = float(np.sum(off ** 2))

    red = np.sum([r["red"] for r in results], axis=0, dtype=np.float64)
    mse_sum = float(red[:, 0].sum())
    ln_sum = float(red[:, 1].sum())

    num_pixels = N * HI * WI
    mse_loss = mse_sum / (NI * CI * HI * WI)
    bpp_loss = ln_sum / (-math.log(2) * num_pixels)
    loss = LMBDA * 255.0 ** 2 * mse_loss + bpp_loss + LMBDA_CORR * corr_loss
    return np.float32(loss)


# revision 4
# speedup vs baseline: 1.1471x; 1.0133x over previous
"""Trainium2 Bass kernel for BatchChannelDecorrelationLoss.

Contract: kernel(**inputs) takes FULL unsharded inputs
  y:             (16, 192, 32, 32) f32
  x_hat:         (16, 3, 512, 512) f32
  target:        (16, 3, 512, 512) f32
  likelihoods_y: (16, 192, 32, 32) f32
and returns the FULL output: scalar f32 loss.

Strategy (data-parallel over batch N across 8 cores, 2 samples/core):
  device, per core:
    - per-(n,c) max / min of y over H*W (f32, exact)   -> stats (384, 2)
    - row-Gram B = Z^T Z over all 384 (n,c) rows, bf16 -> b0/b1/b2 tiles
      (upper block-triangle; host extracts the two per-sample 192x192
       diagonal blocks; bf16 is fine: corr term is ~1e-6 of the loss)
    - row sums via ones-vector matmul                  -> rs (1, 384)
    - sum((x_hat-target)^2) partial per partition      -> red[:, 0]
    - sum(log(lik)) partial per partition              -> red[:, 1]
  host:
    - rates = sum_n (round(max) - round(min))  [round commutes with max/min]
    - stable argsort -> top-64 channel idx  (matches jnp.argsort tie-break)
    - cov = (G_k - S_k S_k^T / M) / (M-1) on the selected 64x64 block
    - loss = lmbda*255^2*mse + bpp + lmbda_corr*sum(offdiag(cov)^2)

DMA layout: all heavy loads issued first (y, lik, then x_hat on the sync
HWDGE queue and target on the scalar HWDGE queue); all stores go through
the gpsimd SWDGE queue so they can never head-of-line-block the loads.
"""

import math
import sys

if "/opt/trn_rl_repo" not in sys.path:
    sys.path.insert(0, "/opt/trn_rl_repo")

import numpy as np

import concourse.bacc as bacc
import concourse.masks as masks
import concourse.mybir as mybir
import concourse.tile as tile
from concourse.bass_utils import run_bass_kernel_spmd

# ---- problem constants (hardcoded per spec) ----
N, C, HY, WY = 16, 192, 32, 32
NI, CI, HI, WI = 16, 3, 512, 512
TOP_K = 64
LMBDA = 0.01
LMBDA_CORR = 1e-4
N_CORES = 8
NS = N // N_CORES          # samples per core = 2
YROWS = NS * C             # 384
YCOLS = HY * WY            # 1024
MSE_COLS = NS * CI * HI * WI // 128   # 12288
LIK_COLS = NS * C * HY * WY // 128    # 3072
MSE_CHUNK = 2048
N_MSE = MSE_COLS // MSE_CHUNK         # 6
NJ = YCOLS // 128                     # 8 hw chunks

FP32 = mybir.dt.float32
BF16 = mybir.dt.bfloat16
AX = mybir.AxisListType
OP = mybir.AluOpType
AF = mybir.ActivationFunctionType

_prog_cache = {}


def _build_program():
    nc = bacc.Bacc("TRN2", target_bir_lowering=False, debug=False,
                   num_devices=N_CORES)

    ys = nc.dram_tensor("ys", [YROWS, YCOLS], FP32, kind="ExternalInput")
    xh = nc.dram_tensor("xh", [128, MSE_COLS], FP32, kind="ExternalInput")
    tg = nc.dram_tensor("tg", [128, MSE_COLS], FP32, kind="ExternalInput")
    lk = nc.dram_tensor("lk", [128, LIK_COLS], FP32, kind="ExternalInput")

    stats = nc.dram_tensor("stats", [YROWS, 2], FP32, kind="ExternalOutput")
    b0 = nc.dram_tensor("b0", [128, 256], FP32, kind="ExternalOutput")
    b1 = nc.dram_tensor("b1", [128, 256], FP32, kind="ExternalOutput")
    b2 = nc.dram_tensor("b2", [128, 128], FP32, kind="ExternalOutput")
    rs = nc.dram_tensor("rs", [1, YROWS], FP32, kind="ExternalOutput")
    red = nc.dram_tensor("red", [128, 2], FP32, kind="ExternalOutput")

    with tile.TileContext(nc) as tc:
        with (
            tc.tile_pool(name="singles", bufs=1) as singles,
            tc.tile_pool(name="ypool", bufs=3) as ypool,
            tc.tile_pool(name="ybf", bufs=3) as ybfp,
            tc.tile_pool(name="ztp", bufs=8) as ztp,
            tc.tile_pool(name="stp", bufs=3) as stp,
            tc.tile_pool(name="mx", bufs=N_MSE) as mxp,
            tc.tile_pool(name="mt", bufs=N_MSE) as mtp,
            tc.tile_pool(name="lkp", bufs=1) as lkp,
            tc.tile_pool(name="tpsum", bufs=4, space="PSUM") as tpsum,
            tc.tile_pool(name="gpsum", bufs=1, space="PSUM") as gpsum,
        ):
            # ---- phase 0: all heavy loads, in issue order ----
            ytiles = []
            for t in range(3):
                yt = ypool.tile([128, YCOLS], FP32, tag="yt")
                nc.gpsimd.dma_start(yt[:], ys[t * 128:(t + 1) * 128, :])
                ytiles.append(yt)
            lt = lkp.tile([128, LIK_COLS], FP32)
            nc.gpsimd.dma_start(lt[:], lk[:])
            mse_x, mse_t = [], []
            for i in range(N_MSE):
                sl = slice(i * MSE_CHUNK, (i + 1) * MSE_CHUNK)
                xt = mxp.tile([128, MSE_CHUNK], FP32, tag="xt")
                nc.sync.dma_start(xt[:], xh[:, sl])
                tt = mtp.tile([128, MSE_CHUNK], FP32, tag="tt")
                nc.scalar.dma_start(tt[:], tg[:, sl])
                mse_x.append(xt)
                mse_t.append(tt)

            ident = singles.tile([128, 128], BF16)
            masks.make_identity(nc, ident[:])
            ones = singles.tile([128, 1], BF16)
            nc.gpsimd.memset(ones[:], 1.0)

            # ---- likelihoods: sum(log(lk)) partials per partition ----
            redsb = singles.tile([128, 2], FP32)
            nc.scalar.activation(lt[:], lt[:], AF.Ln,
                                 accum_out=redsb[:, 1:2])

            # ---- y: stats + bf16 cast ----
            ybf = []
            for t in range(3):
                st = stp.tile([128, 2], FP32, tag="st")
                nc.vector.tensor_reduce(st[:, 0:1], ytiles[t][:], axis=AX.X,
                                        op=OP.max)
                nc.vector.tensor_reduce(st[:, 1:2], ytiles[t][:], axis=AX.X,
                                        op=OP.min)
                nc.gpsimd.dma_start(stats[t * 128:(t + 1) * 128, :], st[:])
                yb = ybfp.tile([128, YCOLS], BF16, tag="yb")
                nc.vector.tensor_copy(yb[:], ytiles[t][:])
                ybf.append(yb)

            # ---- transpose to Z tiles: 8 x (128 hw, 384 rows) bf16 ----
            zts = []
            for j in range(NJ):
                sl = slice(j * 128, (j + 1) * 128)
                zt = ztp.tile([128, YROWS], BF16, tag="zt")
                for t in range(3):
                    pt = tpsum.tile([128, 128], BF16, tag="tp")
                    nc.tensor.transpose(pt[:], ybf[t][:, sl], ident[:])
                    nc.vector.tensor_copy(zt[:, t * 128:(t + 1) * 128],
                                          pt[:])
                zts.append(zt)

            # ---- row-Gram upper blocks + row sums, PSUM-accumulated ----
            pb0 = gpsum.tile([128, 256], FP32, tag="pb0")
            for j, zt in enumerate(zts):
                nc.tensor.matmul(pb0[:], lhsT=zt[:, 0:128], rhs=zt[:, 0:256],
                                 start=(j == 0), stop=(j == NJ - 1))
            pb1 = gpsum.tile([128, 256], FP32, tag="pb1")
            for j, zt in enumerate(zts):
                nc.tensor.matmul(pb1[:], lhsT=zt[:, 128:256],
                                 rhs=zt[:, 128:384],
                                 start=(j == 0), stop=(j == NJ - 1))
            pb2 = gpsum.tile([128, 128], FP32, tag="pb2")
            for j, zt in enumerate(zts):
                nc.tensor.matmul(pb2[:], lhsT=zt[:, 256:384],
                                 rhs=zt[:, 256:384],
                                 start=(j == 0), stop=(j == NJ - 1))
            prs = gpsum.tile([1, YROWS], FP32, tag="prs")
            for j, zt in enumerate(zts):
                nc.tensor.matmul(prs[:], lhsT=ones[:], rhs=zt[:],
                                 start=(j == 0), stop=(j == NJ - 1))

            for psum_t, dram_t, w in ((pb0, b0, 256), (pb1, b1, 256),
                                      (pb2, b2, 128)):
                sb = singles.tile([128, w], FP32, tag=f"sb{w}",
                                  name=f"gout_{dram_t.name}")
                nc.vector.tensor_copy(sb[:], psum_t[:])
                nc.gpsimd.dma_start(dram_t[:], sb[:])
            rssb = singles.tile([1, YROWS], FP32)
            nc.vector.tensor_copy(rssb[:], prs[:])
            nc.gpsimd.dma_start(rs[:], rssb[:])

            # ---- MSE: sum((xh - tg)^2), partials per partition ----
            macc = singles.tile([128, N_MSE], FP32)
            for i in range(N_MSE):
                xt, tt = mse_x[i], mse_t[i]
                nc.vector.tensor_tensor(xt[:], xt[:], tt[:], op=OP.subtract)
                nc.scalar.activation(xt[:], xt[:], AF.Square,
                                     accum_out=macc[:, i:i + 1])

            nc.vector.tensor_reduce(redsb[:, 0:1], macc[:], axis=AX.X,
                                    op=OP.add)
            nc.gpsimd.dma_start(red[:], redsb[:])

    nc.compile()
    return nc


def _get_program():
    if "nc" not in _prog_cache:
        _prog_cache["nc"] = _build_program()
    return _prog_cache["nc"]


def kernel(y, x_hat, target, likelihoods_y):
    y = np.ascontiguousarray(y, dtype=np.float32)
    x_hat = np.ascontiguousarray(x_hat, dtype=np.float32)
    target = np.ascontiguousarray(target, dtype=np.float32)
    lik = np.ascontiguousarray(likelihoods_y, dtype=np.float32)

    nc = _get_program()

    in_maps = []
    for c in range(N_CORES):
        s = slice(c * NS, (c + 1) * NS)
        in_maps.append({
            "ys": y[s].reshape(YROWS, YCOLS),
            "xh": x_hat[s].reshape(128, MSE_COLS),
            "tg": target[s].reshape(128, MSE_COLS),
            "lk": lik[s].reshape(128, LIK_COLS),
        })

    res = run_bass_kernel_spmd(nc, in_maps, list(range(N_CORES)))
    results = res.results

    # ---- host-side combine (all O(C^2) and smaller) ----
    stats = np.stack([r["stats"] for r in results])       # (8, 384, 2)
    stats = stats.reshape(N, C, 2)
    fmax, fmin = stats[..., 0], stats[..., 1]

    # rates: round commutes with max/min; np.round == jnp.round (half-to-even)
    per_sample = np.round(fmax).astype(np.int64) - np.round(fmin).astype(np.int64)
    rates = per_sample.sum(axis=0)                        # (192,)
    idx = np.argsort(rates, kind="stable")[::-1][:TOP_K]

    # row-Gram -> per-channel Gram G and sums S
    B = np.zeros((YROWS, YROWS), dtype=np.float64)
    for r in results:
        B[0:128, 0:256] += r["b0"]
        B[128:256, 128:384] += r["b1"]
        B[256:384, 256:384] += r["b2"]
    B = np.triu(B) + np.triu(B, 1).T
    G = B[0:C, 0:C] + B[C:2 * C, C:2 * C]

    rs_all = np.sum([r["rs"] for r in results], axis=0,
                    dtype=np.float64).reshape(YROWS)
    S = rs_all[0:C] + rs_all[C:2 * C]

    M = N * HY * WY                                       # 16384
    Gk = G[np.ix_(idx, idx)]
    Sk = S[idx]
    cov = (Gk - np.outer(Sk, Sk) / M) / (M - 1)
    off = cov - np.diag(np.diag(cov))
    corr_loss = float(np.sum(off ** 2))

    red = np.sum([r["red"] for r in results], axis=0, dtype=np.float64)
    mse_sum = float(red[:, 0].sum())
    ln_sum = float(red[:, 1].sum())

    num_pixels = N * HI * WI
    mse_loss = mse_sum / (NI * CI * HI * WI)
    bpp_loss = ln_sum / (-math.log(2) * num_pixels)
    loss = LMBDA * 255.0 ** 2 * mse_loss + bpp_loss + LMBDA_CORR * corr_loss
    return np.float32(loss)
